# revision 31
# baseline (speedup 1.0000x reference)
"""Windowed (Swin-style) multi-head attention on 8 TRN2 NeuronCores.

Data-parallel: 256 independent windows -> 32 per core. Per window:
  qkv = x @ w_qkv ; per-head attn = softmax(q k^T * scale + bias) ; out = (attn v) @ w_proj + b_proj

Device-side layout strategy (all matmuls contract over the partition dim):
  - host pre-transposes x to channel-major xT[c, tok] so qT/kT are produced
    feature-major (ready to be score-matmul operands) and v token-major.
  - scores are computed TRANSPOSED, S^T[k, q] (lhsT = kT tile, rhs = qT), so
    softmax normalization runs over the partition axis: exp via ScalarE
    (scale folded in), * exp(bias) via VectorE, column-sums via ones-block
    matmul on TensorE broadcast onto the avT feature rows,
    reciprocal_approx_fast + normalize on VectorE.
  - avT[f, q] = v-contracted matmul accumulated over k tiles; normalized
    avT is exactly the lhsT the projection matmul needs. b_proj is added
    (pre-broadcast on host) during the PSUM->SBUF output copy.
Performance structure (measured on HW):
  - all K=64 score matmuls issue as adjacent (even,odd)-head pairs on
    disjoint PE row groups, and rowsum/avT matmuls as adjacent pairs on
    disjoint column groups -> each pair streams concurrently (2x).
  - 2-bank PSUM quad tiles feed single 1024-wide ScalarE/VectorE ops
    (each DVE op pays a ~280-cycle fixed cost, so fewer/wider wins).
  - per head-pair step, qk chains are emitted right before the score
    quads that consume them, so EXPs sit early in the ScalarE FIFO and
    the score PSUM ring recycles without stalling the PE.
  - head DMAs are spread over the sync/scalar/gpsimd queues and junk
    warm-up matmuls bridge the HAM cold window.
Matmul operands are bf16 (full-rate PE, fp32 PSUM accumulation); inputs are
rounded to bf16 on the host so they can be DMA'd directly.
"""

import sys

for _p in ("/opt/trn_rl_repo",):
    if _p not in sys.path:
        sys.path.insert(0, _p)

import ml_dtypes
import numpy as np
from contextlib import ExitStack

import concourse.bass as bass
import concourse.bacc as bacc
import concourse.mybir as mybir
from concourse import tile
from concourse.bass_utils import run_bass_kernel_spmd

NCORES = 8
BS = 256
W = BS // NCORES  # windows per core
N = 256           # tokens per window
DIM = 512
NH = 8
HD = 64
SCALE = HD ** -0.5
F32 = mybir.dt.float32
BF16 = mybir.dt.bfloat16
NPBF = ml_dtypes.bfloat16
EXP = mybir.ActivationFunctionType.Exp
COPY = mybir.ActivationFunctionType.Copy


def build(w_count=W):
    nc = bacc.Bacc(None, target_bir_lowering=False)
    xt = nc.declare_dram_parameter("xt", [w_count, DIM, N], BF16, False)
    wqk = nc.declare_dram_parameter("wqk", [DIM, 2 * DIM], BF16, False)
    wv = nc.declare_dram_parameter("wv", [DIM, DIM], BF16, False)
    wp = nc.declare_dram_parameter("wp", [DIM, DIM], BF16, False)
    brep = nc.declare_dram_parameter("brep", [128, 2 * DIM], F32, False)
    ebt = nc.declare_dram_parameter("ebt", [128, 2 * NH * N], BF16, False)
    ones_c = nc.declare_dram_parameter("ones_c", [128, HD], BF16, False)
    out = nc.declare_dram_parameter("out", [w_count, N, DIM], F32, True)

    with ExitStack() as ctx:
        tc = ctx.enter_context(tile.TileContext(nc))
        const = ctx.enter_context(tc.tile_pool(name="const", bufs=1))
        p_xt = ctx.enter_context(tc.tile_pool(name="xt", bufs=4))
        p_qk = ctx.enter_context(tc.tile_pool(name="qk", bufs=3))
        p_v = ctx.enter_context(tc.tile_pool(name="v", bufs=3))
        p_p = ctx.enter_context(tc.tile_pool(name="pp", bufs=4))
        p_e = ctx.enter_context(tc.tile_pool(name="te", bufs=4))
        p_av = ctx.enter_context(tc.tile_pool(name="av", bufs=4))
        p_rs = ctx.enter_context(tc.tile_pool(name="rs", bufs=3))
        p_o = ctx.enter_context(tc.tile_pool(name="os", bufs=6))
        # PSUM: one 2-bank quad ring (scores/v/rowsum+avT/proj, 3 bufs =
        # 6 banks) + the qk accumulator ring (2 banks) = 8 banks exactly;
        # the qk ring is isolated so late softmax consumers never gate the
        # dense qk chains.
        ps = ctx.enter_context(tc.tile_pool(name="ps", bufs=2, space="PSUM"))
        psp = ctx.enter_context(tc.tile_pool(name="psp", bufs=3, space="PSUM"))

        # Head loads are spread across four DMA queues (sync/scalar/vector/
        # gpsimd) so the first pair's inputs arrive in ~12us instead of a
        # single queue's ~20us; eb is split per head-pair quarter so the
        # first softmax multiply never waits for the whole 1MB table.
        wj_s = const.tile([128, 512], BF16)
        nc.gpsimd.memset(wj_s[:], 1.0)
        wqk_s = const.tile([128, 4, 2 * DIM], BF16)
        wqk_r = wqk.ap().rearrange("(t p) f -> p t f", p=128)
        nc.scalar.dma_start(wqk_s[:, 0, :], wqk_r[:, 0, :])
        nc.scalar.dma_start(wqk_s[:, 1, :], wqk_r[:, 1, :])
        nc.gpsimd.dma_start(wqk_s[:, 2, :], wqk_r[:, 2, :])
        nc.gpsimd.dma_start(wqk_s[:, 3, :], wqk_r[:, 3, :])
        oc_s = const.tile([128, HD], BF16)
        nc.sync.dma_start(oc_s[:], ones_c.ap())
        eb_s = const.tile([128, 2 * NH * N], BF16)
        ebt_ap = ebt.ap()
        nc.gpsimd.dma_start(eb_s[:, 0:4 * N], ebt_ap[:, 0:4 * N])
        wv_s = const.tile([128, 4, DIM], BF16)
        nc.gpsimd.dma_start(wv_s[:], wv.ap().rearrange("(t p) f -> p t f", p=128))
        for hp in range(1, 4):
            nc.gpsimd.dma_start(
                eb_s[:, hp * 4 * N:(hp + 1) * 4 * N],
                ebt_ap[:, hp * 4 * N:(hp + 1) * 4 * N],
            )
        wp_s = const.tile([128, 4, DIM], BF16)
        br_s = const.tile([128, 2 * DIM], F32)
        nc.gpsimd.dma_start(br_s[:], brep.ap())

        # Junk matmuls while the head DMAs stream: keeps the PE busy
        # through the HAM activity window so the first real matmuls run at
        # full clock instead of the cold half-rate.
        wu = psp.tile([128, 4, N], F32, tag="scp", name="warm")
        for _ in range(12):
            nc.tensor.matmul(wu[:, 0:2, :], wj_s[:, 0:128], wj_s[:],
                             start=True, stop=True)

        xt_ap = xt.ap()
        out_ap = out.ap()

        # pp column index for (head, ktile): per head-pair the layout is
        # (e_k0, e_k1, o_k0, o_k1); score matmuls are ISSUED interleaved
        # e_k0, o_k0, e_k1, o_k1 so adjacent matmuls hit disjoint PE row
        # groups (and rowsum/avT orderings hit disjoint col groups).
        def ppi(h, kt):
            return (h // 2) * 4 + (h % 2) * 2 + kt

        for wp2 in range(w_count // 2):
            w0 = 2 * wp2
            # load xT (channel-major) for both windows: [128, ct, win, tok]
            xt_s = p_xt.tile([128, 4, 2, N], BF16, tag="xt")
            for wl in range(2):
                nc.sync.dma_start(
                    xt_s[:, :, wl, :],
                    xt_ap[w0 + wl].rearrange("(t p) q -> p t q", p=128),
                )
            if wp2 == 0:
                # wp rides the sync queue right behind the first xt pair:
                # off the gpsimd queue's critical path, arrives well before
                # the first projection.
                nc.sync.dma_start(
                    wp_s[:], wp.ap().rearrange("(t p) f -> p t f", p=128)
                )

            qk_s = p_qk.tile([128, 8, 2, N], BF16, tag="qk")
            # per-window v tiles (separate so the two copies can run on
            # different engines without a cross-engine same-tile edge)
            v_w = [
                p_v.tile([128, 2, DIM], BF16, tag="v", name="v_0"),
                p_v.tile([128, 2, DIM], BF16, tag="v", name="v_1"),
            ]
            pp = [
                p_p.tile([128, 2 * NH * N], BF16, tag="pp", name="pp_0"),
                p_p.tile([128, 2 * NH * N], BF16, tag="pp", name="pp_1"),
            ]

            def qk_chain(ft):
                # qkT[feat, (win tok)] batched over the window pair (N=512
                # keeps LDWEIGHTS hidden behind the matmul)
                acc = ps.tile([128, 512], F32, tag="ps")
                for ct in range(4):
                    nc.tensor.matmul(
                        acc[:],
                        wqk_s[:, ct, ft * 128:(ft + 1) * 128],
                        xt_s[:, ct, :, :],
                        start=(ct == 0),
                        stop=(ct == 3),
                    )
                nc.scalar.activation(
                    qk_s[:, ft, :, :].rearrange("p a q -> p (a q)"), acc[:], COPY
                )

            def v_chain(wi):
                # per window one 2-bank quad accumulator (kt chains in
                # separate banks) -> a single 1024-wide PSUM->SBUF copy;
                # one copy on ScalarE, one on DVE to balance both queues
                vq = psp.tile([128, 4, N], F32, tag="scp",
                              name=f"vq_{wi}")
                for kt in range(2):
                    for ct in range(4):
                        nc.tensor.matmul(
                            vq[:, 2 * kt:2 * kt + 2, :],
                            xt_s[:, ct, wi, kt * 128:(kt + 1) * 128],
                            wv_s[:, ct, :],
                            start=(ct == 0),
                            stop=(ct == 3),
                        )
                dst = v_w[wi][:].rearrange("p a q -> p (a q)")
                src = vq[:].rearrange("p a q -> p (a q)")
                if wi == 0:
                    nc.scalar.activation(dst, src, COPY)
                else:
                    nc.vector.tensor_copy(dst, src)

            def score_quad(wi, hp):
                # scores^T -> exp(scale*s) * exp(bias) -> pp.
                # Per head-pair one 2-bank quad tile [E_k0|E_k1|O_k0|O_k1]:
                # MMs issued (E,O) alternating so every adjacent pair hits
                # disjoint PE row groups and runs packed; the quad feeds ONE
                # 1024-wide EXP + ONE 1024-wide bias multiply.
                scp = psp.tile([128, 4, N], F32, tag="scp")
                for kt in range(2):
                    for s in range(2):
                        base = s * HD
                        nc.tensor.matmul(
                            scp[:, s * 2 + kt, :],
                            qk_s[base:base + HD, 4 + hp, wi,
                                 kt * 128:(kt + 1) * 128],
                            qk_s[base:base + HD, hp, wi, :],
                            start=True,
                            stop=True,
                        )
                te = p_e.tile([128, 4 * N], BF16, tag="te",
                              name=f"te_{wi}_{hp}")
                nc.scalar.activation(
                    te[:], scp[:].rearrange("p a q -> p (a q)"),
                    EXP, scale=SCALE,
                )
                off = hp * 4 * N
                nc.vector.tensor_mul(
                    pp[wi][:, off:off + 4 * N],
                    te[:],
                    eb_s[:, off:off + 4 * N],
                )

            def phase_ra(wi, pp_s):
                # fused rowsum + avT per feature-tile pair: the ones-block
                # rowsum MM (denominators broadcast onto the 64 partitions
                # its head's avT features occupy) is issued back-to-back
                # with the avT MM of the OTHER head of the pair, so every
                # adjacent MM alternates PE column group and runs packed.
                av_s = p_av.tile([128, 4 * N], BF16, tag="av",
                                 name=f"av_{wi}")
                # two quad tiles per window: ALL rowsums in rq (quarter =
                # feature tile), ALL avT accumulators in aq -> one batched
                # reciprocal + one batched normalize (1024-wide DVE ops)
                rq = psp.tile([128, 4, N], F32, tag="scp", name=f"rq_{wi}")
                aq = psp.tile([128, 4, N], F32, tag="scp", name=f"aq_{wi}")
                for ftl in range(4):
                    for z in range(2):
                        ha = 2 * ftl + z        # avT head (cols z*HD)
                        hr = 2 * ftl + 1 - z    # rowsum head (other col grp)
                        for kt in range(2):
                            nc.tensor.matmul(
                                aq[z * HD:(z + 1) * HD, ftl, :],
                                v_w[wi][:, kt, ha * HD:(ha + 1) * HD],
                                pp_s[:, ppi(ha, kt) * N:(ppi(ha, kt) + 1) * N],
                                start=(kt == 0),
                                stop=(kt == 1),
                                tile_position=(0, z * HD),
                            )
                            nc.tensor.matmul(
                                rq[(1 - z) * HD:(2 - z) * HD, ftl, :],
                                oc_s[:],
                                pp_s[:, ppi(hr, kt) * N:(ppi(hr, kt) + 1) * N],
                                start=(kt == 0),
                                stop=(kt == 1),
                                tile_position=(0, (1 - z) * HD),
                            )
                rb = p_rs.tile([128, 4 * N], F32, tag="rb",
                               name=f"rb_{wi}")
                nc.vector.reciprocal_approx_fast(
                    rb[:], rq[:].rearrange("p a q -> p (a q)")
                )
                nc.vector.tensor_mul(
                    av_s[:],
                    aq[:].rearrange("p a q -> p (a q)"),
                    rb[:],
                )
                return av_s

            def phase_proj(wi, av_s):
                # projection into one 2-bank quad (qt chains per bank), one
                # batched b_proj add during the PSUM->SBUF copy, one output
                # DMA per window on the gpsimd queue
                w = w0 + wi
                oq = psp.tile([128, 4, N], F32, tag="scp",
                              name=f"oq_{wi}")
                for qt in range(2):
                    for ftl in range(4):
                        nc.tensor.matmul(
                            oq[:, 2 * qt:2 * qt + 2, :],
                            av_s[:, ftl * N + qt * 128:ftl * N + qt * 128 + 128],
                            wp_s[:, ftl, :],
                            start=(ftl == 0),
                            stop=(ftl == 3),
                        )
                o_s = p_o.tile([128, 2, DIM], F32, tag="os",
                               name=f"os_{wi}")
                nc.vector.tensor_add(
                    o_s[:].rearrange("p a q -> p (a q)"),
                    oq[:].rearrange("p a q -> p (a q)"),
                    br_s[:],
                )
                nc.gpsimd.dma_start(
                    out_ap[w].rearrange("(qt p) f -> p qt f", p=128), o_s[:]
                )

            # per head-pair step: 2 qk chains then both windows' score
            # quads -- each quad's EXP lands 2-3 slots into the ScalarE
            # FIFO (instead of behind all 8 copies), so the score-PSUM
            # ring recycles early; v chains slot in after steps 0/1 so the
            # v copies complete before the RA phases need them.
            for hp in range(4):
                qk_chain(hp)
                qk_chain(4 + hp)
                score_quad(0, hp)
                score_quad(1, hp)
                if hp == 0:
                    v_chain(0)
                elif hp == 1:
                    v_chain(1)
            av0 = phase_ra(0, pp[0])
            phase_proj(0, av0)
            av1 = phase_ra(1, pp[1])
            phase_proj(1, av1)

    nc.finalize()
    return nc


_NC_CACHE = {}


def _get_nc(w_count):
    if w_count not in _NC_CACHE:
        _NC_CACHE[w_count] = build(w_count)
    return _NC_CACHE[w_count]


def _prep(inputs, w_count):
    x = np.asarray(inputs["x"], dtype=np.float32)
    noise = np.asarray(inputs["noise"], dtype=np.float32)
    ns = np.asarray(inputs["noise_strength"], dtype=np.float32)
    wqkv = np.asarray(inputs["w_qkv"], dtype=np.float32)
    wproj = np.asarray(inputs["w_proj"], dtype=np.float32)
    bproj = np.asarray(inputs["b_proj"], dtype=np.float32)
    bt = np.asarray(inputs["bias_table"], dtype=np.float32)
    ri = np.asarray(inputs["rel_index"])

    xe = x + noise * ns                                     # [BS, N, DIM]
    xt = np.ascontiguousarray(xe.transpose(0, 2, 1).astype(NPBF))
    eb = np.exp(bt[ri])                                     # [q, k, h]
    ebT = eb.transpose(2, 1, 0)                             # [h, k, q]
    ebt = np.zeros((128, 2 * NH, N), np.float32)
    for h in range(NH):
        for kt in range(2):
            i = (h // 2) * 4 + (h % 2) * 2 + kt
            ebt[:, i, :] = ebT[h, kt * 128:(kt + 1) * 128, :]
    ebt = np.ascontiguousarray(ebt.reshape(128, 2 * NH * N).astype(NPBF))
    common = {
        "wqk": np.ascontiguousarray(wqkv[:, : 2 * DIM].astype(NPBF)),
        "wv": np.ascontiguousarray(wqkv[:, 2 * DIM:].astype(NPBF)),
        "wp": np.ascontiguousarray(wproj.astype(NPBF)),
        "brep": np.ascontiguousarray(
            np.broadcast_to(
                np.tile(bproj, 2).reshape(1, 2 * DIM), (128, 2 * DIM)
            ).astype(np.float32)
        ),
        "ebt": ebt,
        "ones_c": np.ones((128, HD), NPBF),
    }
    in_maps = []
    for i in range(NCORES):
        m = dict(common)
        m["xt"] = np.ascontiguousarray(xt[i * w_count:(i + 1) * w_count])
        in_maps.append(m)
    return in_maps


def _run(inputs, w_count=W, trace=False, trace_cores=None):
    nc = _get_nc(w_count)
    in_maps = _prep(inputs, w_count)
    kw = {}
    if trace_cores is not None:
        kw["trace_cores"] = trace_cores
    res = run_bass_kernel_spmd(
        nc, in_maps, core_ids=list(range(NCORES)), trace=trace, **kw
    )
    full = np.concatenate([res.results[i]["out"] for i in range(NCORES)], axis=0)
    return full, res


def kernel(**inputs):
    out, _ = _run(inputs, W, trace=False)
    return out


def kernel_profiled(inputs, w_count=W, trace_cores=None):
    out, res = _run(inputs, w_count, trace=True, trace_cores=trace_cores)
    return out, res

